# revision 2
# baseline (speedup 1.0000x reference)
"""EventDenoisingMamba Trainium2 kernel, v2.

Data-parallel over batch: 8 batch elements -> 8 NeuronCores. Channels on
partitions, time on the free dimension.

v2 structural changes vs v1:
  - softplus path: delta = -ln(sigmoid(-(x+dtb))) -- 2 ACT ops instead of
    4 ACT + 1 DVE add. The sign is folded through the scan (h' = -h) and
    fixed up in the final (u*D) - S' scalar_tensor_tensor.
  - one scan per state n covering BOTH d-blocks: [db0 tc | 2-col reset
    gap | db1 tc]. da=0 in the gap kills the carry across the boundary;
    db1's initial state is injected into the gap's dbu column by an ACT
    copy. Halves scan-instruction count; no DVE carry casts (carries are
    ACT copies into per-n persistent tiles).
  - dbu on GpSimd, ymult on DVE, y-sum via accumulate-DMAs (SWDGE) or a
    DVE/GpSimd pair tree (USE_ACCUM_DMA switch).
"""

import numpy as np

import concourse.bass as bass
import concourse.tile as tile
from concourse import bacc, mybir

F32 = mybir.dt.float32
BF16 = mybir.dt.bfloat16
AF = mybir.ActivationFunctionType
OP = mybir.AluOpType

S = 8192
DM = 128      # d_model
DI = 256      # d_inner
NST = 16      # d_state
DC = 4        # d_conv
RK = 8        # dt_rank
NL = 4        # layers
NCORES = 8

USE_ACCUM_DMA = True
# which n run their dbu multiply on gpsimd (rest on vector)
DBU_GP = set(range(16))
# which n run their ymult on gpsimd (rest on vector)
YM_GP = set()


class Ctx:
    pass


def _load_weights(c, nc, drams):
    wp = c.wp
    (kuc, wz, xpw, dtw, ow, emb, headw, dtbn, cb, apos, dpar, embb,
     headb, featT) = drams
    c.w_kuc, c.w_wz, c.w_xpw, c.w_dtw, c.w_ow = [], [], [], [], []
    c.w_dtbn, c.w_cb, c.w_a, c.w_d = [], [], [], []
    for l in range(NL):
        for lst, dram, shape, dt in [
            (c.w_kuc, kuc, [128, DC * DI], BF16),
            (c.w_wz, wz, [128, DI], BF16),
            (c.w_xpw, xpw, [128, 80], BF16),
            (c.w_dtw, dtw, [RK, DI], BF16),
            (c.w_ow, ow, [128, 256], BF16),
            (c.w_dtbn, dtbn, [128, 2], F32),
            (c.w_cb, cb, [128, 2], F32),
            (c.w_a, apos, [128, 2 * NST], F32),
            (c.w_d, dpar, [128, 2], F32),
        ]:
            t = wp.tile(shape, dt, tag=f"w{len(lst)}_{id(lst) % 997}",
                        name=f"w{len(lst)}_{id(lst) % 997}")
            nc.sync.dma_start(t, dram[l])
            lst.append(t)
    c.w_emb = wp.tile([11, DM], BF16, tag="emb", name="emb")
    nc.sync.dma_start(c.w_emb, emb[:])
    c.w_headw = wp.tile([DM, 1], BF16, tag="headw", name="headw")
    nc.sync.dma_start(c.w_headw, headw[:])
    c.w_embb = wp.tile([128, 1], F32, tag="embb", name="embb")
    nc.sync.dma_start(c.w_embb, embb[:])
    c.w_headb = wp.tile([1, 1], F32, tag="headb", name="headb")
    nc.sync.dma_start(c.w_headb, headb[:])
    c.w_zero = wp.tile([128, 1], F32, tag="zero", name="zero")
    nc.vector.memset(c.w_zero, 0.0)
    c.w_eps = wp.tile([128, 1], F32, tag="eps", name="eps")
    nc.vector.memset(c.w_eps, 1e-38)


def _embed(c, nc):
    for blk in range(c.s // c.bw):
        ps = c.pp.tile([128, c.bw], F32, tag="mm", name="mm")
        for h in range(c.bw // 512):
            col = blk * c.bw + h * 512
            nc.tensor.matmul(
                ps[:, h * 512:(h + 1) * 512],
                lhsT=c.w_emb, rhs=c.w_feat[:, col:col + 512],
                start=True, stop=True)
        nc.scalar.activation(
            c.xa[:, 3 + blk * c.bw: 3 + (blk + 1) * c.bw],
            ps, AF.Identity, bias=c.w_embb[:, 0:1])


def _uz(c, nc, l, xin, t0, db, blk):
    bw = c.bw
    ps = c.pp.tile([128, bw], F32, tag="mm", name="mm")
    for h in range(bw // 512):
        col = t0 + blk * bw + h * 512
        for k in range(DC):
            nc.tensor.matmul(
                ps[:, h * 512:(h + 1) * 512],
                lhsT=c.w_kuc[l][:, k * DI + db * 128:k * DI + db * 128 + 128],
                rhs=xin[:, col + k:col + k + 512],
                start=(k == 0), stop=(k == DC - 1))
    off = db * c.tc_len + blk * bw
    nc.scalar.activation(
        c.u_sb[:, off:off + bw], ps, AF.Silu,
        bias=c.w_cb[l][:, db:db + 1])
    ps = c.pp.tile([128, bw], F32, tag="mm", name="mm")
    for h in range(bw // 512):
        col = t0 + blk * bw + h * 512
        nc.tensor.matmul(
            ps[:, h * 512:(h + 1) * 512],
            lhsT=c.w_wz[l][:, db * 128:db * 128 + 128],
            rhs=xin[:, 3 + col:3 + col + 512],
            start=True, stop=True)
    nc.scalar.activation(
        c.zs_sb[:, off:off + bw], ps, AF.Silu)


def _xdbl(c, nc, l, blk):
    bw = c.bw
    ps = c.pp.tile([128, bw], F32, tag="mm", name="mm")
    for h in range(bw // 512):
        col = blk * bw + h * 512
        for ct in range(2):
            nc.tensor.matmul(
                ps[0:40, h * 512:(h + 1) * 512],
                lhsT=c.w_xpw[l][:, ct * 40:ct * 40 + 40],
                rhs=c.u_sb[:, ct * c.tc_len + col:ct * c.tc_len + col + 512],
                start=(ct == 0), stop=(ct == 1))
    nc.scalar.activation(
        c.xd_sb[:, blk * bw:(blk + 1) * bw], ps[0:40, :], AF.Copy)


def _delta(c, nc, l, db, blk):
    """de = ln(sigmoid(-(x+dtb))) = -softplus(x+dtb) = -delta."""
    bw = c.bw
    ps = c.pp.tile([128, bw], F32, tag="mm", name="mm")
    for h in range(bw // 512):
        col = blk * bw + h * 512
        nc.tensor.matmul(
            ps[:, h * 512:(h + 1) * 512],
            lhsT=c.w_dtw[l][:, db * 128:db * 128 + 128],
            rhs=c.xd_sb[0:RK, col:col + 512],
            start=True, stop=True)
    r = c.tmpp.tile([128, bw], F32, tag="tm", name="sig")
    nc.scalar.activation(r, ps, AF.Sigmoid, scale=-1.0,
                         bias=c.w_dtbn[l][:, db:db + 1])
    off = db * c.tc_len + blk * bw
    # +1e-38 bias: the sigmoid table clamps to exactly 0 for very negative
    # inputs; ln(0) = -inf would poison du. Caps delta at ~87.5.
    nc.scalar.activation(c.de_sb[:, off:off + bw], r, AF.Ln,
                         bias=c.w_eps[:, 0:1])


def _ssm(c, nc, l, ci, bcd_r):
    """Scan + y for one chunk. Combined db0|gap|db1 scan per state n."""
    tc = c.tc_len
    t2 = 2 * tc
    tg = t2 + 2          # gapped width
    nc.gpsimd.dma_start(bcd_r, c.xd_sb[RK:RK + 2 * NST, :])
    for n in range(NST):
        bb = c.bcp.tile([128, tc], BF16, tag="bb", name="bb")
        cb2 = c.bcp.tile([128, tc], BF16, tag="cb2", name="cb2")
        for j, (row, dst) in enumerate(((n, bb), (NST + n, cb2))):
            srow = bcd_r[row:row + 1, :]
            bcast = bass.AP(tensor=srow.tensor, offset=srow.offset,
                            ap=[[0, 128]] + [list(x) for x in srow.ap[1:]])
            qeng = (nc.sync, nc.scalar)[j]
            qeng.dma_start(dst, bcast)
        # dbu' = du' * B (both d-blocks; bb read twice via step-0 AP) into
        # the gapped tile: [0:tc] db0, [tc+2:tg] db1.
        dbu_t = c.dbup.tile([128, tg], BF16, tag="dbu", name="dbu")
        dbu_v = bass.AP(tensor=dbu_t.tensor, offset=dbu_t.offset,
                        ap=[list(dbu_t.ap[0]), [tc + 2, 2], [1, tc]])
        du_v = c.du_sb.rearrange("p (b t) -> p b t", b=2)
        bb2 = bass.AP(tensor=bb.tensor, offset=bb.offset,
                      ap=[list(bb.ap[0]), [0, 2]] + [list(x) for x in bb.ap[1:]])
        deng = nc.gpsimd if n in DBU_GP else nc.vector
        deng.tensor_tensor(dbu_v, du_v, bb2, OP.mult)
        # Gap columns [tc, tc+1] <- [db0 carry, db1 carry] (ACT copy casts
        # f32 -> bf16; col tc's value is finite filler, col tc+1 is db1's
        # initial state — da=0 across the gap resets the recurrence).
        # Chunk 0 copies zeros.
        if ci == 0:
            gsrc = bass.AP(tensor=c.w_zero.tensor, offset=c.w_zero.offset,
                           ap=[list(c.w_zero.ap[0]), [0, 2]])
        else:
            gsrc = c.hc[l % 2][n][:, 0:2]
        nc.scalar.activation(dbu_t[:, tc:tc + 2], gsrc, AF.Copy)
        # da = exp(|A| * de) = exp(-|A| delta), per db half; zero the gap.
        da_t = c.dap.tile([128, tg], F32, tag="da", name="da")
        zsrc = bass.AP(tensor=c.w_zero.tensor, offset=c.w_zero.offset,
                       ap=[list(c.w_zero.ap[0]), [0, 2]])
        nc.scalar.activation(da_t[:, tc:tc + 2], zsrc, AF.Copy)
        nc.scalar.activation(da_t[:, 0:tc], c.de_sb[:, 0:tc], AF.Exp,
                             scale=c.w_a[l][:, n:n + 1])
        nc.scalar.activation(da_t[:, tc + 2:tg], c.de_sb[:, tc:t2], AF.Exp,
                             scale=c.w_a[l][:, NST + n:NST + n + 1])
        h_t = c.hp.tile([128, tg], BF16, tag="h", name="h")
        init = 0.0 if ci == 0 else c.hc[l % 2][n][:, 0:1]
        nc.vector.tensor_tensor_scan(
            h_t, da_t, dbu_t, initial=init, op0=OP.mult, op1=OP.add)
        # carry out (cols tc-1 and tg-1) for the next chunk, via ACT
        if ci < c.nch - 1:
            hsrc = bass.AP(tensor=h_t.tensor, offset=h_t.offset + tc - 1,
                           ap=[list(h_t.ap[0]), [tc + 2, 2]])
            nc.scalar.activation(c.hc[l % 2][n], hsrc, AF.Copy)
        # y'_n = h' * C ; accumulate into y_sb
        cc2 = bass.AP(tensor=cb2.tensor, offset=cb2.offset,
                      ap=[list(cb2.ap[0]), [0, 2]] + [list(x) for x in cb2.ap[1:]])
        h_v = bass.AP(tensor=h_t.tensor, offset=h_t.offset,
                      ap=[list(h_t.ap[0]), [tc + 2, 2], [1, tc]])
        yeng = nc.gpsimd if n in YM_GP else nc.vector
        if USE_ACCUM_DMA:
            if n == 0:
                yeng.tensor_tensor(
                    c.y_sb.rearrange("p (b t) -> p b t", b=2), h_v, cc2,
                    OP.mult)
            else:
                yt = c.ytp.tile([128, t2], BF16, tag="yt", name="yt")
                yeng.tensor_tensor(
                    yt.rearrange("p (b t) -> p b t", b=2), h_v, cc2, OP.mult)
                nc.gpsimd.dma_start(c.y_sb, yt, accum_op=OP.add)
        else:
            yt = c.ytp.tile([128, t2], BF16, tag="yt", name="yt", bufs=6)
            yeng.tensor_tensor(
                yt.rearrange("p (b t) -> p b t", b=2), h_v, cc2, OP.mult)
            if n == 0:
                c.y_list = [yt]
            else:
                c.y_list.append(yt)
    if not USE_ACCUM_DMA:
        # pair tree on vector/gpsimd
        lv = c.y_list
        rot = 0
        while len(lv) > 1:
            nx = []
            for i in range(0, len(lv) - 1, 2):
                o = c.ytp.tile([128, t2], BF16, tag="yt", name="yt", bufs=6)
                eng = nc.gpsimd if rot % 3 == 2 else nc.vector
                eng.tensor_tensor(o, lv[i], lv[i + 1], OP.add)
                nx.append(o)
                rot += 1
            if len(lv) % 2:
                nx.append(lv[-1])
            lv = nx
        c.y_sum = lv[0]


def _outproj(c, nc, l, xout, t0, yg, blk):
    bw = c.bw
    ps = c.pp.tile([128, bw], F32, tag="mm", name="mm")
    for h in range(bw // 512):
        col = blk * bw + h * 512
        for ct in range(2):
            nc.tensor.matmul(
                ps[:, h * 512:(h + 1) * 512],
                lhsT=c.w_ow[l][:, ct * 128:ct * 128 + 128],
                rhs=yg[:, ct * c.tc_len + col:ct * c.tc_len + col + 512],
                start=(ct == 0), stop=(ct == 1))
    nc.scalar.activation(
        xout[:, 3 + t0 + blk * bw:3 + t0 + (blk + 1) * bw], ps, AF.Copy)


def _layer(c, nc, l, bcd):
    xin = c.xa if l % 2 == 0 else c.xb
    xout = c.xb if l % 2 == 0 else c.xa
    t2 = 2 * c.tc_len
    for ci in range(c.nch):
        t0 = ci * c.tc_len
        c.u_sb = c.ubufp.tile([128, t2], BF16, tag="u", name="u")
        c.zs_sb = c.zbufp.tile([128, t2], BF16, tag="z", name="z")
        c.de_sb = c.dbufp.tile([128, t2], BF16, tag="de", name="de")
        c.du_sb = c.dubufp.tile([128, t2], BF16, tag="du", name="du")
        c.y_sb = c.ybufp.tile([128, t2], BF16, tag="y", name="y")
        c.xd_sb = c.xdblp.tile([40, c.tc_len], BF16, tag="xd", name="xd")

        for db in range(2):
            for blk in range(c.nblk):
                _uz(c, nc, l, xin, t0, db, blk)
        for blk in range(c.nblk):
            _xdbl(c, nc, l, blk)
        for db in range(2):
            for blk in range(c.nblk):
                _delta(c, nc, l, db, blk)
        nc.gpsimd.tensor_tensor(c.du_sb, c.de_sb, c.u_sb, OP.mult)

        _ssm(c, nc, l, ci, bcd[(l * c.nch + ci) % 4])

        ysum = c.y_sb if USE_ACCUM_DMA else c.y_sum
        yg = c.ygatep.tile([128, t2], BF16, tag="yg", name="yg")
        yf = c.ygatep.tile([128, t2], BF16, tag="yf", name="yf")
        for db in range(2):
            sl = slice(db * c.tc_len, (db + 1) * c.tc_len)
            # yf = (u * D) - S'  (S' = -sum_n C h)
            nc.vector.scalar_tensor_tensor(
                yf[:, sl], c.u_sb[:, sl], c.w_d[l][:, db:db + 1],
                ysum[:, sl], OP.mult, OP.subtract)
        nc.vector.tensor_tensor(yg, yf, c.zs_sb, OP.mult)
        for blk in range(c.nblk):
            _outproj(c, nc, l, xout, t0, yg, blk)


def _head(c, nc, out):
    xfin = c.xa if NL % 2 == 0 else c.xb
    for blk in range(c.s // c.bw):
        ps = c.pp.tile([128, c.bw], F32, tag="mm", name="mm")
        for h in range(c.bw // 512):
            col = blk * c.bw + h * 512
            nc.tensor.matmul(
                ps[0:1, h * 512:(h + 1) * 512],
                lhsT=c.w_headw, rhs=xfin[:, 3 + col:3 + col + 512],
                start=True, stop=True)
        ot = c.tmpp.tile([128, c.bw], F32, tag="tm", name="ot")
        nc.scalar.activation(ot[0:1, :], ps[0:1, :], AF.Sigmoid,
                             bias=c.w_headb[0:1, 0:1])
        nc.sync.dma_start(out[0:1, blk * c.bw:(blk + 1) * c.bw], ot[0:1, :])


def build(s=S, tc_len=1024, nloops=1):
    nc = bacc.Bacc("TRN2", target_bir_lowering=False, debug=False,
                   num_devices=NCORES)
    drams = (
        nc.declare_dram_parameter("kuc", [NL, 128, DC * DI], BF16, False),
        nc.declare_dram_parameter("wz", [NL, 128, DI], BF16, False),
        nc.declare_dram_parameter("xpw", [NL, 128, 80], BF16, False),
        nc.declare_dram_parameter("dtw", [NL, RK, DI], BF16, False),
        nc.declare_dram_parameter("ow", [NL, 128, 256], BF16, False),
        nc.declare_dram_parameter("emb", [11, DM], BF16, False),
        nc.declare_dram_parameter("headw", [DM, 1], BF16, False),
        nc.declare_dram_parameter("dtbn", [NL, 128, 2], F32, False),
        nc.declare_dram_parameter("cb", [NL, 128, 2], F32, False),
        nc.declare_dram_parameter("apos", [NL, 128, 2 * NST], F32, False),
        nc.declare_dram_parameter("dpar", [NL, 128, 2], F32, False),
        nc.declare_dram_parameter("embb", [128, 1], F32, False),
        nc.declare_dram_parameter("headb", [1, 1], F32, False),
    )
    featT = nc.declare_dram_parameter("featT", [11, s], BF16, False)
    out = nc.declare_dram_parameter("out", [1, s], F32, True)
    bcd = nc.dram_tensor("bcd", [4, 2 * NST, tc_len], BF16)
    drams = tuple(list(drams) + [featT])

    c = Ctx()
    c.s = s
    c.tc_len = tc_len
    c.nch = s // tc_len
    c.bw = min(tc_len, 1024)
    c.nblk = tc_len // c.bw
    c.da_bufs = 2
    c.da_seen = 0

    with tile.TileContext(nc) as tcx:
        with (
            tcx.tile_pool(name="w", bufs=1) as wp,
            tcx.tile_pool(name="psP", bufs=4, space="PSUM") as pp,
            tcx.tile_pool(name="bcast", bufs=4) as bcp,
        ):
            c.wp, c.pp, c.bcp = wp, pp, bcp
            _load_weights(c, nc, drams)
            c.xa = wp.tile([128, 3 + s], BF16, tag="xa", name="xa")
            c.xb = wp.tile([128, 3 + s], BF16, tag="xb", name="xb")
            nc.vector.memset(c.xa[:, 0:3], 0.0)
            nc.vector.memset(c.xb[:, 0:3], 0.0)
            # per-n carry tiles, double-banked across layers (l % 2)
            c.hc = [[wp.tile([128, 2], F32, tag=f"hc{b}_{n}",
                             name=f"hc{b}_{n}") for n in range(NST)]
                    for b in range(2)]

            with tcx.tile_pool(name="feat", bufs=1) as fp:
                c.w_feat = fp.tile([11, s], BF16, tag="featT", name="featT")
                nc.sync.dma_start(c.w_feat, drams[13][:])
                _embed(c, nc)

            with (
                tcx.tile_pool(name="ubuf", bufs=2) as ubufp,
                tcx.tile_pool(name="zbuf", bufs=2) as zbufp,
                tcx.tile_pool(name="dbuf", bufs=2) as dbufp,
                tcx.tile_pool(name="dubuf", bufs=2) as dubufp,
                tcx.tile_pool(name="xdbl", bufs=2) as xdblp,
                tcx.tile_pool(name="ybuf", bufs=2) as ybufp,
                tcx.tile_pool(name="ygate", bufs=2) as ygatep,
                tcx.tile_pool(name="da", bufs=2) as dap,
                tcx.tile_pool(name="dbu", bufs=3) as dbup,
                tcx.tile_pool(name="hb", bufs=3) as hp,
                tcx.tile_pool(name="yt", bufs=4) as ytp,
                tcx.tile_pool(name="tmp", bufs=2) as tmpp,
            ):
                c.ubufp, c.zbufp, c.dbufp, c.dubufp = ubufp, zbufp, dbufp, dubufp
                c.xdblp, c.ybufp, c.ygatep = xdblp, ybufp, ygatep
                c.dap, c.dbup, c.hp, c.ytp, c.tmpp = dap, dbup, hp, ytp, tmpp

                for rep in range(nloops):
                    for l in range(NL):
                        _layer(c, nc, l, bcd)
                _head(c, nc, out)

    nc.compile()
    return nc


_CACHE = {}


def _get_nc(s, tc_len, nloops=1):
    key = (s, tc_len, nloops)
    if key not in _CACHE:
        _CACHE[key] = build(s, tc_len, nloops)
    return _CACHE[key]


def prep_inputs(features, emb_w, emb_b, in_proj_w, conv_w, conv_b, x_proj_w,
                dt_w, dt_b, A_log, D, out_proj_w, head_w, head_b):
    """Host-side weight preprocessing shared by all cores."""
    import ml_dtypes
    f32 = np.float32
    bf16 = ml_dtypes.bfloat16

    nl = in_proj_w.shape[0]
    kuc = np.zeros((nl, 128, DC * DI), dtype=f32)
    for l in range(nl):
        wu = in_proj_w[l][:, :DI]                      # [128, 256]
        for k in range(DC):
            kuc[l][:, k * DI:(k + 1) * DI] = wu * conv_w[l][:, k][None, :]
    wz = in_proj_w[:, :, DI:]                          # [NL, 128, 256]
    xpw = np.zeros((nl, 128, 80), dtype=f32)
    ow = np.zeros((nl, 128, 256), dtype=f32)
    apos = np.zeros((nl, 128, 2 * NST), dtype=f32)
    dtbn = np.zeros((nl, 128, 2), dtype=f32)
    cb2 = np.zeros((nl, 128, 2), dtype=f32)
    dp2 = np.zeros((nl, 128, 2), dtype=f32)
    for l in range(nl):
        for ct in range(2):
            xpw[l][:, ct * 40:(ct + 1) * 40] = \
                x_proj_w[l][ct * 128:(ct + 1) * 128, :]
            ow[l][:, ct * 128:(ct + 1) * 128] = \
                out_proj_w[l][ct * 128:(ct + 1) * 128, :]
            apos[l][:, ct * NST:(ct + 1) * NST] = \
                np.exp(A_log[l][ct * 128:(ct + 1) * 128, :])
            dtbn[l][:, ct] = -dt_b[l][ct * 128:(ct + 1) * 128]
            cb2[l][:, ct] = conv_b[l][ct * 128:(ct + 1) * 128]
            dp2[l][:, ct] = D[l][ct * 128:(ct + 1) * 128]

    return {
        "kuc": kuc.astype(bf16),
        "wz": np.ascontiguousarray(wz).astype(bf16),
        "xpw": xpw.astype(bf16),
        "dtw": np.ascontiguousarray(dt_w).astype(bf16),
        "ow": ow.astype(bf16),
        "emb": np.ascontiguousarray(emb_w).astype(bf16),
        "headw": np.ascontiguousarray(head_w).astype(bf16),
        "dtbn": dtbn,
        "cb": cb2,
        "apos": apos,
        "dpar": dp2,
        "embb": np.asarray(emb_b).reshape(128, 1).astype(f32),
        "headb": np.asarray(head_b).reshape(1, 1).astype(f32),
    }


def kernel(features, emb_w, emb_b, in_proj_w, conv_w, conv_b, x_proj_w,
           dt_w, dt_b, A_log, D, out_proj_w, head_w, head_b,
           _tc_len=1024, _trace=False):
    from concourse.bass_utils import run_bass_kernel_spmd
    import ml_dtypes

    args = [np.asarray(a) for a in (
        features, emb_w, emb_b, in_proj_w, conv_w, conv_b, x_proj_w,
        dt_w, dt_b, A_log, D, out_proj_w, head_w, head_b)]
    features = args[0]
    b, s, _ = features.shape
    assert b == NCORES
    nc = _get_nc(s, _tc_len)
    common = prep_inputs(*args)
    in_maps = []
    for i in range(NCORES):
        m = dict(common)
        m["featT"] = np.ascontiguousarray(
            features[i].T).astype(ml_dtypes.bfloat16)
        in_maps.append(m)
    res = run_bass_kernel_spmd(nc, in_maps, core_ids=list(range(NCORES)),
                               trace=_trace)
    out = np.stack([r["out"].reshape(s, 1) for r in res.results])
    kernel.last_result = res
    return out.astype(np.float32)


# revision 3
# speedup vs baseline: 1.0371x; 1.0371x over previous
"""EventDenoisingMamba Trainium2 kernel, v2.

Data-parallel over batch: 8 batch elements -> 8 NeuronCores. Channels on
partitions, time on the free dimension.

v2 structural changes vs v1:
  - softplus path: delta = -ln(sigmoid(-(x+dtb))) -- 2 ACT ops instead of
    4 ACT + 1 DVE add. The sign is folded through the scan (h' = -h) and
    fixed up in the final (u*D) - S' scalar_tensor_tensor.
  - one scan per state n covering BOTH d-blocks: [db0 tc | 2-col reset
    gap | db1 tc]. da=0 in the gap kills the carry across the boundary;
    db1's initial state is injected into the gap's dbu column by an ACT
    copy. Halves scan-instruction count; no DVE carry casts (carries are
    ACT copies into per-n persistent tiles).
  - dbu on GpSimd, ymult on DVE, y-sum via accumulate-DMAs (SWDGE) or a
    DVE/GpSimd pair tree (USE_ACCUM_DMA switch).
"""

import numpy as np

import concourse.bass as bass
import concourse.tile as tile
from concourse import bacc, mybir

F32 = mybir.dt.float32
BF16 = mybir.dt.bfloat16
AF = mybir.ActivationFunctionType
OP = mybir.AluOpType

S = 8192
DM = 128      # d_model
DI = 256      # d_inner
NST = 16      # d_state
DC = 4        # d_conv
RK = 8        # dt_rank
NL = 4        # layers
NCORES = 8

USE_ACCUM_DMA = True
# which n run their dbu multiply on gpsimd (rest on vector)
DBU_GP = set(range(16))
# which n run their ymult on gpsimd (rest on vector)
YM_GP = set()


class Ctx:
    pass


def _load_weights(c, nc, drams):
    wp = c.wp
    (kuc, wz, xpw, dtw, ow, emb, headw, dtbn, cb, apos, dpar, embb,
     headb, featT) = drams
    c.w_kuc, c.w_wz, c.w_xpw, c.w_dtw, c.w_ow = [], [], [], [], []
    c.w_dtbn, c.w_cb, c.w_a, c.w_d = [], [], [], []
    for l in range(NL):
        for lst, dram, shape, dt in [
            (c.w_kuc, kuc, [128, DC * DI], BF16),
            (c.w_wz, wz, [128, DI], BF16),
            (c.w_xpw, xpw, [128, 80], BF16),
            (c.w_dtw, dtw, [RK, DI], BF16),
            (c.w_ow, ow, [128, 256], BF16),
            (c.w_dtbn, dtbn, [128, 2], F32),
            (c.w_cb, cb, [128, 2], F32),
            (c.w_a, apos, [128, 2 * NST], F32),
            (c.w_d, dpar, [128, 2], F32),
        ]:
            t = wp.tile(shape, dt, tag=f"w{len(lst)}_{id(lst) % 997}",
                        name=f"w{len(lst)}_{id(lst) % 997}")
            nc.sync.dma_start(t, dram[l])
            lst.append(t)
    c.w_emb = wp.tile([11, DM], BF16, tag="emb", name="emb")
    nc.sync.dma_start(c.w_emb, emb[:])
    c.w_headw = wp.tile([DM, 1], BF16, tag="headw", name="headw")
    nc.sync.dma_start(c.w_headw, headw[:])
    c.w_embb = wp.tile([128, 1], F32, tag="embb", name="embb")
    nc.sync.dma_start(c.w_embb, embb[:])
    c.w_headb = wp.tile([1, 1], F32, tag="headb", name="headb")
    nc.sync.dma_start(c.w_headb, headb[:])
    c.w_zero = wp.tile([128, 1], F32, tag="zero", name="zero")
    nc.vector.memset(c.w_zero, 0.0)
    c.w_eps = wp.tile([128, 1], F32, tag="eps", name="eps")
    nc.vector.memset(c.w_eps, 1e-38)


def _embed(c, nc):
    for blk in range(c.s // c.bw):
        ps = c.pp.tile([128, c.bw], F32, tag="mm", name="mm")
        for h in range(c.bw // 512):
            col = blk * c.bw + h * 512
            nc.tensor.matmul(
                ps[:, h * 512:(h + 1) * 512],
                lhsT=c.w_emb, rhs=c.w_feat[:, col:col + 512],
                start=True, stop=True)
        nc.scalar.activation(
            c.xa[:, 3 + blk * c.bw: 3 + (blk + 1) * c.bw],
            ps, AF.Identity, bias=c.w_embb[:, 0:1])


def _uz(c, nc, l, xin, t0, db, blk):
    bw = c.bw
    ps = c.pp.tile([128, bw], F32, tag="mm", name="mm")
    for h in range(bw // 512):
        col = t0 + blk * bw + h * 512
        for k in range(DC):
            nc.tensor.matmul(
                ps[:, h * 512:(h + 1) * 512],
                lhsT=c.w_kuc[l][:, k * DI + db * 128:k * DI + db * 128 + 128],
                rhs=xin[:, col + k:col + k + 512],
                start=(k == 0), stop=(k == DC - 1))
    off = db * c.tc_len + blk * bw
    nc.scalar.activation(
        c.u_sb[:, off:off + bw], ps, AF.Silu,
        bias=c.w_cb[l][:, db:db + 1])
    ps = c.pp.tile([128, bw], F32, tag="mm", name="mm")
    for h in range(bw // 512):
        col = t0 + blk * bw + h * 512
        nc.tensor.matmul(
            ps[:, h * 512:(h + 1) * 512],
            lhsT=c.w_wz[l][:, db * 128:db * 128 + 128],
            rhs=xin[:, 3 + col:3 + col + 512],
            start=True, stop=True)
    nc.scalar.activation(
        c.zs_sb[:, off:off + bw], ps, AF.Silu)


def _xdbl(c, nc, l, blk):
    bw = c.bw
    ps = c.pp.tile([128, bw], F32, tag="mm", name="mm")
    for h in range(bw // 512):
        col = blk * bw + h * 512
        for ct in range(2):
            nc.tensor.matmul(
                ps[0:40, h * 512:(h + 1) * 512],
                lhsT=c.w_xpw[l][:, ct * 40:ct * 40 + 40],
                rhs=c.u_sb[:, ct * c.tc_len + col:ct * c.tc_len + col + 512],
                start=(ct == 0), stop=(ct == 1))
    nc.scalar.activation(
        c.xd_sb[:, blk * bw:(blk + 1) * bw], ps[0:40, :], AF.Copy)


def _delta(c, nc, l, db, blk):
    """de = ln(sigmoid(-(x+dtb))) = -softplus(x+dtb) = -delta."""
    bw = c.bw
    ps = c.pp.tile([128, bw], F32, tag="mm", name="mm")
    for h in range(bw // 512):
        col = blk * bw + h * 512
        nc.tensor.matmul(
            ps[:, h * 512:(h + 1) * 512],
            lhsT=c.w_dtw[l][:, db * 128:db * 128 + 128],
            rhs=c.xd_sb[0:RK, col:col + 512],
            start=True, stop=True)
    r = c.tmpp.tile([128, bw], F32, tag="tm", name="sig")
    nc.scalar.activation(r, ps, AF.Sigmoid, scale=-1.0,
                         bias=c.w_dtbn[l][:, db:db + 1])
    off = db * c.tc_len + blk * bw
    # +1e-38 bias: the sigmoid table clamps to exactly 0 for very negative
    # inputs; ln(0) = -inf would poison du. Caps delta at ~87.5.
    nc.scalar.activation(c.de_sb[:, off:off + bw], r, AF.Ln,
                         bias=c.w_eps[:, 0:1])


def _ssm(c, nc, l, ci, bcd_r):
    """Scan + y for one chunk (two scans per state n, contiguous tiles)."""
    tc = c.tc_len
    t2 = 2 * tc
    nc.gpsimd.dma_start(bcd_r, c.xd_sb[RK:RK + 2 * NST, :])
    for n in range(NST):
        bb = c.bcp.tile([128, tc], BF16, tag="bb", name="bb")
        cb2 = c.bcp.tile([128, tc], BF16, tag="cb2", name="cb2")
        for j, (row, dst) in enumerate(((n, bb), (NST + n, cb2))):
            srow = bcd_r[row:row + 1, :]
            bcast = bass.AP(tensor=srow.tensor, offset=srow.offset,
                            ap=[[0, 128]] + [list(x) for x in srow.ap[1:]])
            qeng = (nc.sync, nc.scalar)[j]
            qeng.dma_start(dst, bcast)
        # dbu' = du' * B (both d-blocks in one op; bb read twice via
        # step-0 AP)
        dbu_t = c.dbup.tile([128, t2], BF16, tag="dbu", name="dbu")
        bb2 = bass.AP(tensor=bb.tensor, offset=bb.offset,
                      ap=[list(bb.ap[0]), [0, 2]] + [list(x) for x in bb.ap[1:]])
        deng = nc.gpsimd if n in DBU_GP else nc.vector
        deng.tensor_tensor(
            dbu_t.rearrange("p (b t) -> p b t", b=2),
            c.du_sb.rearrange("p (b t) -> p b t", b=2), bb2, OP.mult)
        # da = exp(|A| * de) = exp(-|A| delta), per db half
        da_t = c.dap.tile([128, t2], F32, tag="da", name="da")
        for db in range(2):
            nc.scalar.activation(
                da_t[:, db * tc:(db + 1) * tc],
                c.de_sb[:, db * tc:(db + 1) * tc], AF.Exp,
                scale=c.w_a[l][:, db * NST + n:db * NST + n + 1])
        h_t = c.hp.tile([128, t2], BF16, tag="h", name="h")
        for db in range(2):
            init = 0.0 if ci == 0 else c.hc[l % 2][n][:, db:db + 1]
            nc.vector.tensor_tensor_scan(
                h_t[:, db * tc:(db + 1) * tc],
                da_t[:, db * tc:(db + 1) * tc],
                dbu_t[:, db * tc:(db + 1) * tc],
                initial=init, op0=OP.mult, op1=OP.add)
        # carry out (cols tc-1 and t2-1) for the next chunk, via ACT
        if ci < c.nch - 1:
            hsrc = bass.AP(tensor=h_t.tensor, offset=h_t.offset + tc - 1,
                           ap=[list(h_t.ap[0]), [tc, 2]])
            nc.scalar.activation(c.hc[l % 2][n], hsrc, AF.Copy)
        # y'_n = h' * C ; accumulate into y_sb
        cc2 = bass.AP(tensor=cb2.tensor, offset=cb2.offset,
                      ap=[list(cb2.ap[0]), [0, 2]] + [list(x) for x in cb2.ap[1:]])
        yeng = nc.gpsimd if n in YM_GP else nc.vector
        if USE_ACCUM_DMA:
            if n == 0:
                yeng.tensor_tensor(
                    c.y_sb.rearrange("p (b t) -> p b t", b=2),
                    h_t.rearrange("p (b t) -> p b t", b=2), cc2, OP.mult)
            else:
                yt = c.ytp.tile([128, t2], BF16, tag="yt", name="yt")
                yeng.tensor_tensor(
                    yt.rearrange("p (b t) -> p b t", b=2),
                    h_t.rearrange("p (b t) -> p b t", b=2), cc2, OP.mult)
                nc.gpsimd.dma_start(c.y_sb, yt, accum_op=OP.add)
        else:
            yt = c.ytp.tile([128, t2], BF16, tag="yt", name="yt", bufs=6)
            yeng.tensor_tensor(
                yt.rearrange("p (b t) -> p b t", b=2),
                h_t.rearrange("p (b t) -> p b t", b=2), cc2, OP.mult)
            if n == 0:
                c.y_list = [yt]
            else:
                c.y_list.append(yt)
    if not USE_ACCUM_DMA:
        # pair tree on vector/gpsimd
        lv = c.y_list
        rot = 0
        while len(lv) > 1:
            nx = []
            for i in range(0, len(lv) - 1, 2):
                o = c.ytp.tile([128, t2], BF16, tag="yt", name="yt", bufs=6)
                eng = nc.gpsimd if rot % 3 == 2 else nc.vector
                eng.tensor_tensor(o, lv[i], lv[i + 1], OP.add)
                nx.append(o)
                rot += 1
            if len(lv) % 2:
                nx.append(lv[-1])
            lv = nx
        c.y_sum = lv[0]


def _outproj(c, nc, l, xout, t0, yg, blk):
    bw = c.bw
    ps = c.pp.tile([128, bw], F32, tag="mm", name="mm")
    for h in range(bw // 512):
        col = blk * bw + h * 512
        for ct in range(2):
            nc.tensor.matmul(
                ps[:, h * 512:(h + 1) * 512],
                lhsT=c.w_ow[l][:, ct * 128:ct * 128 + 128],
                rhs=yg[:, ct * c.tc_len + col:ct * c.tc_len + col + 512],
                start=(ct == 0), stop=(ct == 1))
    nc.scalar.activation(
        xout[:, 3 + t0 + blk * bw:3 + t0 + (blk + 1) * bw], ps, AF.Copy)


def _layer(c, nc, l, bcd):
    xin = c.xa if l % 2 == 0 else c.xb
    xout = c.xb if l % 2 == 0 else c.xa
    t2 = 2 * c.tc_len
    for ci in range(c.nch):
        t0 = ci * c.tc_len
        c.u_sb = c.ubufp.tile([128, t2], BF16, tag="u", name="u")
        c.zs_sb = c.zbufp.tile([128, t2], BF16, tag="z", name="z")
        c.de_sb = c.dbufp.tile([128, t2], BF16, tag="de", name="de")
        c.du_sb = c.dubufp.tile([128, t2], BF16, tag="du", name="du")
        c.y_sb = c.ybufp.tile([128, t2], BF16, tag="y", name="y")
        c.xd_sb = c.xdblp.tile([40, c.tc_len], BF16, tag="xd", name="xd")

        for db in range(2):
            for blk in range(c.nblk):
                _uz(c, nc, l, xin, t0, db, blk)
        for blk in range(c.nblk):
            _xdbl(c, nc, l, blk)
        for db in range(2):
            for blk in range(c.nblk):
                _delta(c, nc, l, db, blk)
        nc.gpsimd.tensor_tensor(c.du_sb, c.de_sb, c.u_sb, OP.mult)

        _ssm(c, nc, l, ci, bcd[(l * c.nch + ci) % 4])

        ysum = c.y_sb if USE_ACCUM_DMA else c.y_sum
        yg = c.ygatep.tile([128, t2], BF16, tag="yg", name="yg")
        yf = c.ygatep.tile([128, t2], BF16, tag="yf", name="yf")
        for db in range(2):
            sl = slice(db * c.tc_len, (db + 1) * c.tc_len)
            # yf = (u * D) - S'  (S' = -sum_n C h)
            nc.vector.scalar_tensor_tensor(
                yf[:, sl], c.u_sb[:, sl], c.w_d[l][:, db:db + 1],
                ysum[:, sl], OP.mult, OP.subtract)
        nc.vector.tensor_tensor(yg, yf, c.zs_sb, OP.mult)
        for blk in range(c.nblk):
            _outproj(c, nc, l, xout, t0, yg, blk)


def _head(c, nc, out):
    xfin = c.xa if NL % 2 == 0 else c.xb
    for blk in range(c.s // c.bw):
        ps = c.pp.tile([128, c.bw], F32, tag="mm", name="mm")
        for h in range(c.bw // 512):
            col = blk * c.bw + h * 512
            nc.tensor.matmul(
                ps[0:1, h * 512:(h + 1) * 512],
                lhsT=c.w_headw, rhs=xfin[:, 3 + col:3 + col + 512],
                start=True, stop=True)
        ot = c.tmpp.tile([128, c.bw], F32, tag="tm", name="ot")
        nc.scalar.activation(ot[0:1, :], ps[0:1, :], AF.Sigmoid,
                             bias=c.w_headb[0:1, 0:1])
        nc.sync.dma_start(out[0:1, blk * c.bw:(blk + 1) * c.bw], ot[0:1, :])


def build(s=S, tc_len=1024, nloops=1):
    nc = bacc.Bacc("TRN2", target_bir_lowering=False, debug=False,
                   num_devices=NCORES)
    drams = (
        nc.declare_dram_parameter("kuc", [NL, 128, DC * DI], BF16, False),
        nc.declare_dram_parameter("wz", [NL, 128, DI], BF16, False),
        nc.declare_dram_parameter("xpw", [NL, 128, 80], BF16, False),
        nc.declare_dram_parameter("dtw", [NL, RK, DI], BF16, False),
        nc.declare_dram_parameter("ow", [NL, 128, 256], BF16, False),
        nc.declare_dram_parameter("emb", [11, DM], BF16, False),
        nc.declare_dram_parameter("headw", [DM, 1], BF16, False),
        nc.declare_dram_parameter("dtbn", [NL, 128, 2], F32, False),
        nc.declare_dram_parameter("cb", [NL, 128, 2], F32, False),
        nc.declare_dram_parameter("apos", [NL, 128, 2 * NST], F32, False),
        nc.declare_dram_parameter("dpar", [NL, 128, 2], F32, False),
        nc.declare_dram_parameter("embb", [128, 1], F32, False),
        nc.declare_dram_parameter("headb", [1, 1], F32, False),
    )
    featT = nc.declare_dram_parameter("featT", [11, s], BF16, False)
    out = nc.declare_dram_parameter("out", [1, s], F32, True)
    bcd = nc.dram_tensor("bcd", [4, 2 * NST, tc_len], BF16)
    drams = tuple(list(drams) + [featT])

    c = Ctx()
    c.s = s
    c.tc_len = tc_len
    c.nch = s // tc_len
    c.bw = min(tc_len, 1024)
    c.nblk = tc_len // c.bw
    c.da_bufs = 2
    c.da_seen = 0

    with tile.TileContext(nc) as tcx:
        with (
            tcx.tile_pool(name="w", bufs=1) as wp,
            tcx.tile_pool(name="psP", bufs=4, space="PSUM") as pp,
            tcx.tile_pool(name="bcast", bufs=4) as bcp,
        ):
            c.wp, c.pp, c.bcp = wp, pp, bcp
            _load_weights(c, nc, drams)
            c.xa = wp.tile([128, 3 + s], BF16, tag="xa", name="xa")
            c.xb = wp.tile([128, 3 + s], BF16, tag="xb", name="xb")
            nc.vector.memset(c.xa[:, 0:3], 0.0)
            nc.vector.memset(c.xb[:, 0:3], 0.0)
            # per-n carry tiles, double-banked across layers (l % 2)
            c.hc = [[wp.tile([128, 2], F32, tag=f"hc{b}_{n}",
                             name=f"hc{b}_{n}") for n in range(NST)]
                    for b in range(2)]

            with tcx.tile_pool(name="feat", bufs=1) as fp:
                c.w_feat = fp.tile([11, s], BF16, tag="featT", name="featT")
                nc.sync.dma_start(c.w_feat, drams[13][:])
                _embed(c, nc)

            with (
                tcx.tile_pool(name="ubuf", bufs=2) as ubufp,
                tcx.tile_pool(name="zbuf", bufs=2) as zbufp,
                tcx.tile_pool(name="dbuf", bufs=2) as dbufp,
                tcx.tile_pool(name="dubuf", bufs=2) as dubufp,
                tcx.tile_pool(name="xdbl", bufs=2) as xdblp,
                tcx.tile_pool(name="ybuf", bufs=2) as ybufp,
                tcx.tile_pool(name="ygate", bufs=2) as ygatep,
                tcx.tile_pool(name="da", bufs=2) as dap,
                tcx.tile_pool(name="dbu", bufs=3) as dbup,
                tcx.tile_pool(name="hb", bufs=3) as hp,
                tcx.tile_pool(name="yt", bufs=4) as ytp,
                tcx.tile_pool(name="tmp", bufs=2) as tmpp,
            ):
                c.ubufp, c.zbufp, c.dbufp, c.dubufp = ubufp, zbufp, dbufp, dubufp
                c.xdblp, c.ybufp, c.ygatep = xdblp, ybufp, ygatep
                c.dap, c.dbup, c.hp, c.ytp, c.tmpp = dap, dbup, hp, ytp, tmpp

                for rep in range(nloops):
                    for l in range(NL):
                        _layer(c, nc, l, bcd)
                _head(c, nc, out)

    nc.compile()
    return nc


_CACHE = {}


def _get_nc(s, tc_len, nloops=1):
    key = (s, tc_len, nloops)
    if key not in _CACHE:
        _CACHE[key] = build(s, tc_len, nloops)
    return _CACHE[key]


def prep_inputs(features, emb_w, emb_b, in_proj_w, conv_w, conv_b, x_proj_w,
                dt_w, dt_b, A_log, D, out_proj_w, head_w, head_b):
    """Host-side weight preprocessing shared by all cores."""
    import ml_dtypes
    f32 = np.float32
    bf16 = ml_dtypes.bfloat16

    nl = in_proj_w.shape[0]
    kuc = np.zeros((nl, 128, DC * DI), dtype=f32)
    for l in range(nl):
        wu = in_proj_w[l][:, :DI]                      # [128, 256]
        for k in range(DC):
            kuc[l][:, k * DI:(k + 1) * DI] = wu * conv_w[l][:, k][None, :]
    wz = in_proj_w[:, :, DI:]                          # [NL, 128, 256]
    xpw = np.zeros((nl, 128, 80), dtype=f32)
    ow = np.zeros((nl, 128, 256), dtype=f32)
    apos = np.zeros((nl, 128, 2 * NST), dtype=f32)
    dtbn = np.zeros((nl, 128, 2), dtype=f32)
    cb2 = np.zeros((nl, 128, 2), dtype=f32)
    dp2 = np.zeros((nl, 128, 2), dtype=f32)
    for l in range(nl):
        for ct in range(2):
            xpw[l][:, ct * 40:(ct + 1) * 40] = \
                x_proj_w[l][ct * 128:(ct + 1) * 128, :]
            ow[l][:, ct * 128:(ct + 1) * 128] = \
                out_proj_w[l][ct * 128:(ct + 1) * 128, :]
            apos[l][:, ct * NST:(ct + 1) * NST] = \
                np.exp(A_log[l][ct * 128:(ct + 1) * 128, :])
            dtbn[l][:, ct] = -dt_b[l][ct * 128:(ct + 1) * 128]
            cb2[l][:, ct] = conv_b[l][ct * 128:(ct + 1) * 128]
            dp2[l][:, ct] = D[l][ct * 128:(ct + 1) * 128]

    return {
        "kuc": kuc.astype(bf16),
        "wz": np.ascontiguousarray(wz).astype(bf16),
        "xpw": xpw.astype(bf16),
        "dtw": np.ascontiguousarray(dt_w).astype(bf16),
        "ow": ow.astype(bf16),
        "emb": np.ascontiguousarray(emb_w).astype(bf16),
        "headw": np.ascontiguousarray(head_w).astype(bf16),
        "dtbn": dtbn,
        "cb": cb2,
        "apos": apos,
        "dpar": dp2,
        "embb": np.asarray(emb_b).reshape(128, 1).astype(f32),
        "headb": np.asarray(head_b).reshape(1, 1).astype(f32),
    }


def kernel(features, emb_w, emb_b, in_proj_w, conv_w, conv_b, x_proj_w,
           dt_w, dt_b, A_log, D, out_proj_w, head_w, head_b,
           _tc_len=1024, _trace=False):
    from concourse.bass_utils import run_bass_kernel_spmd
    import ml_dtypes

    args = [np.asarray(a) for a in (
        features, emb_w, emb_b, in_proj_w, conv_w, conv_b, x_proj_w,
        dt_w, dt_b, A_log, D, out_proj_w, head_w, head_b)]
    features = args[0]
    b, s, _ = features.shape
    assert b == NCORES
    nc = _get_nc(s, _tc_len)
    common = prep_inputs(*args)
    in_maps = []
    for i in range(NCORES):
        m = dict(common)
        m["featT"] = np.ascontiguousarray(
            features[i].T).astype(ml_dtypes.bfloat16)
        in_maps.append(m)
    res = run_bass_kernel_spmd(nc, in_maps, core_ids=list(range(NCORES)),
                               trace=_trace)
    out = np.stack([r["out"].reshape(s, 1) for r in res.results])
    kernel.last_result = res
    return out.astype(np.float32)


# revision 4
# speedup vs baseline: 1.0426x; 1.0053x over previous
"""EventDenoisingMamba Trainium2 kernel, v2.

Data-parallel over batch: 8 batch elements -> 8 NeuronCores. Channels on
partitions, time on the free dimension.

v2 structural changes vs v1:
  - softplus path: delta = -ln(sigmoid(-(x+dtb))) -- 2 ACT ops instead of
    4 ACT + 1 DVE add. The sign is folded through the scan (h' = -h) and
    fixed up in the final (u*D) - S' scalar_tensor_tensor.
  - one scan per state n covering BOTH d-blocks: [db0 tc | 2-col reset
    gap | db1 tc]. da=0 in the gap kills the carry across the boundary;
    db1's initial state is injected into the gap's dbu column by an ACT
    copy. Halves scan-instruction count; no DVE carry casts (carries are
    ACT copies into per-n persistent tiles).
  - dbu on GpSimd, ymult on DVE, y-sum via accumulate-DMAs (SWDGE) or a
    DVE/GpSimd pair tree (USE_ACCUM_DMA switch).
"""

import numpy as np

import concourse.bass as bass
import concourse.tile as tile
from concourse import bacc, mybir

F32 = mybir.dt.float32
BF16 = mybir.dt.bfloat16
AF = mybir.ActivationFunctionType
OP = mybir.AluOpType

S = 8192
DM = 128      # d_model
DI = 256      # d_inner
NST = 16      # d_state
DC = 4        # d_conv
RK = 8        # dt_rank
NL = 4        # layers
NCORES = 8

USE_ACCUM_DMA = True
# which n run their dbu multiply on gpsimd (rest on vector)
DBU_GP = set(range(16))
# which n run their ymult on gpsimd (rest on vector)
YM_GP = set()


class Ctx:
    pass


def _load_weights(c, nc, drams):
    wp = c.wp
    (kuc, wz, xpw, dtw, ow, emb, headw, dtbn, cb, apos, dpar, embb,
     headb, featT) = drams
    c.w_kuc, c.w_wz, c.w_xpw, c.w_dtw, c.w_ow = [], [], [], [], []
    c.w_dtbn, c.w_cb, c.w_a, c.w_d = [], [], [], []
    for l in range(NL):
        for lst, dram, shape, dt in [
            (c.w_kuc, kuc, [128, DC * DI], BF16),
            (c.w_wz, wz, [128, DI], BF16),
            (c.w_xpw, xpw, [128, 80], BF16),
            (c.w_dtw, dtw, [RK, DI], BF16),
            (c.w_ow, ow, [128, 256], BF16),
            (c.w_dtbn, dtbn, [128, 2], F32),
            (c.w_cb, cb, [128, 2], F32),
            (c.w_a, apos, [128, 2 * NST], F32),
            (c.w_d, dpar, [128, 2], F32),
        ]:
            t = wp.tile(shape, dt, tag=f"w{len(lst)}_{id(lst) % 997}",
                        name=f"w{len(lst)}_{id(lst) % 997}")
            nc.sync.dma_start(t, dram[l])
            lst.append(t)
    c.w_emb = wp.tile([11, DM], BF16, tag="emb", name="emb")
    nc.sync.dma_start(c.w_emb, emb[:])
    c.w_headw = wp.tile([DM, 1], BF16, tag="headw", name="headw")
    nc.sync.dma_start(c.w_headw, headw[:])
    c.w_embb = wp.tile([128, 1], F32, tag="embb", name="embb")
    nc.sync.dma_start(c.w_embb, embb[:])
    c.w_headb = wp.tile([1, 1], F32, tag="headb", name="headb")
    nc.sync.dma_start(c.w_headb, headb[:])
    c.w_zero = wp.tile([128, 1], F32, tag="zero", name="zero")
    nc.vector.memset(c.w_zero, 0.0)
    c.w_eps = wp.tile([128, 1], F32, tag="eps", name="eps")
    nc.vector.memset(c.w_eps, 1e-38)


def _embed(c, nc):
    for blk in range(c.s // c.bw):
        ps = c.pp.tile([128, c.bw], F32, tag="mm", name="mm")
        for h in range(c.bw // 512):
            col = blk * c.bw + h * 512
            nc.tensor.matmul(
                ps[:, h * 512:(h + 1) * 512],
                lhsT=c.w_emb, rhs=c.w_feat[:, col:col + 512],
                start=True, stop=True)
        nc.scalar.activation(
            c.xa[:, 3 + blk * c.bw: 3 + (blk + 1) * c.bw],
            ps, AF.Identity, bias=c.w_embb[:, 0:1])


def _uz(c, nc, l, xin, t0, db, blk):
    bw = c.bw
    ps = c.pp.tile([128, bw], F32, tag="mm", name="mm")
    for h in range(bw // 512):
        col = t0 + blk * bw + h * 512
        for k in range(DC):
            nc.tensor.matmul(
                ps[:, h * 512:(h + 1) * 512],
                lhsT=c.w_kuc[l][:, k * DI + db * 128:k * DI + db * 128 + 128],
                rhs=xin[:, col + k:col + k + 512],
                start=(k == 0), stop=(k == DC - 1))
    off = db * c.tc_len + blk * bw
    nc.scalar.activation(
        c.u_sb[:, off:off + bw], ps, AF.Silu,
        bias=c.w_cb[l][:, db:db + 1])
    ps = c.pp.tile([128, bw], F32, tag="mm", name="mm")
    for h in range(bw // 512):
        col = t0 + blk * bw + h * 512
        nc.tensor.matmul(
            ps[:, h * 512:(h + 1) * 512],
            lhsT=c.w_wz[l][:, db * 128:db * 128 + 128],
            rhs=xin[:, 3 + col:3 + col + 512],
            start=True, stop=True)
    nc.scalar.activation(
        c.zs_sb[:, off:off + bw], ps, AF.Silu)


def _xdbl(c, nc, l, blk):
    bw = c.bw
    ps = c.pp.tile([128, bw], F32, tag="mm", name="mm")
    for h in range(bw // 512):
        col = blk * bw + h * 512
        for ct in range(2):
            nc.tensor.matmul(
                ps[0:40, h * 512:(h + 1) * 512],
                lhsT=c.w_xpw[l][:, ct * 40:ct * 40 + 40],
                rhs=c.u_sb[:, ct * c.tc_len + col:ct * c.tc_len + col + 512],
                start=(ct == 0), stop=(ct == 1))
    nc.scalar.activation(
        c.xd_sb[:, blk * bw:(blk + 1) * bw], ps[0:40, :], AF.Copy)


def _delta(c, nc, l, db, blk):
    """de = ln(sigmoid(-(x+dtb))) = -softplus(x+dtb) = -delta."""
    bw = c.bw
    ps = c.pp.tile([128, bw], F32, tag="mm", name="mm")
    for h in range(bw // 512):
        col = blk * bw + h * 512
        nc.tensor.matmul(
            ps[:, h * 512:(h + 1) * 512],
            lhsT=c.w_dtw[l][:, db * 128:db * 128 + 128],
            rhs=c.xd_sb[0:RK, col:col + 512],
            start=True, stop=True)
    r = c.tmpp.tile([128, bw], F32, tag="tm", name="sig")
    nc.scalar.activation(r, ps, AF.Sigmoid, scale=-1.0,
                         bias=c.w_dtbn[l][:, db:db + 1])
    off = db * c.tc_len + blk * bw
    # +1e-38 bias: the sigmoid table clamps to exactly 0 for very negative
    # inputs; ln(0) = -inf would poison du. Caps delta at ~87.5.
    nc.scalar.activation(c.de_sb[:, off:off + bw], r, AF.Ln,
                         bias=c.w_eps[:, 0:1])


LAG = 4  # deferral of accum-DMAs / carry copies to avoid head-of-line blocks


def _ssm(c, nc, l, ci, bcd_r):
    """Scan + y for one chunk (two scans per state n, flat per-db TT ops).

    The accumulate-DMA for state n and the carry copy for state n are
    issued LAG states later: an accum-DMA waits on the DVE ymult, and a
    carry copy waits on the scan — issuing them immediately would
    head-of-line-block the Pool / Scalar queues for the next state.
    """
    tc = c.tc_len
    t2 = 2 * tc
    nc.gpsimd.dma_start(bcd_r, c.xd_sb[RK:RK + 2 * NST, :])
    yts = {}
    hts = {}

    def flush_acc(m):
        if USE_ACCUM_DMA and m >= 1:
            nc.gpsimd.dma_start(c.y_sb, yts.pop(m), accum_op=OP.add)

    def flush_carry(m):
        if ci < c.nch - 1 and m >= 0:
            h_t = hts.pop(m)
            hsrc = bass.AP(tensor=h_t.tensor, offset=h_t.offset + tc - 1,
                           ap=[list(h_t.ap[0]), [tc, 2]])
            nc.scalar.activation(c.hc[l % 2][m], hsrc, AF.Copy)

    for n in range(NST):
        bb = c.bcp.tile([128, tc], BF16, tag="bb", name="bb")
        cb2 = c.bcp.tile([128, tc], BF16, tag="cb2", name="cb2")
        for j, (row, dst) in enumerate(((n, bb), (NST + n, cb2))):
            srow = bcd_r[row:row + 1, :]
            bcast = bass.AP(tensor=srow.tensor, offset=srow.offset,
                            ap=[[0, 128]] + [list(x) for x in srow.ap[1:]])
            qeng = (nc.sync, nc.scalar)[j]
            qeng.dma_start(dst, bcast)
        # dbu' = du' * B, one flat op per d-block (step-0 APs run 4x
        # slower on DVE/GpSimd than flat ones)
        dbu_t = c.dbup.tile([128, t2], BF16, tag="dbu", name="dbu")
        deng = nc.gpsimd if n in DBU_GP else nc.vector
        for db in range(2):
            sl = slice(db * tc, (db + 1) * tc)
            deng.tensor_tensor(dbu_t[:, sl], c.du_sb[:, sl], bb, OP.mult)
        # da = exp(|A| * de) = exp(-|A| delta), per db half
        da_t = c.dap.tile([128, t2], F32, tag="da", name="da")
        for db in range(2):
            nc.scalar.activation(
                da_t[:, db * tc:(db + 1) * tc],
                c.de_sb[:, db * tc:(db + 1) * tc], AF.Exp,
                scale=c.w_a[l][:, db * NST + n:db * NST + n + 1])
        h_t = c.hp.tile([128, t2], BF16, tag="h", name="h")
        hts[n] = h_t
        for db in range(2):
            init = 0.0 if ci == 0 else c.hc[l % 2][n][:, db:db + 1]
            nc.vector.tensor_tensor_scan(
                h_t[:, db * tc:(db + 1) * tc],
                da_t[:, db * tc:(db + 1) * tc],
                dbu_t[:, db * tc:(db + 1) * tc],
                initial=init, op0=OP.mult, op1=OP.add)
        # y'_n = h' * C, flat per-db ops
        yeng = nc.gpsimd if n in YM_GP else nc.vector
        if USE_ACCUM_DMA and n == 0:
            yt = c.y_sb
        else:
            yt = c.ytp.tile([128, t2], BF16, tag="yt", name="yt")
            yts[n] = yt
        for db in range(2):
            sl = slice(db * tc, (db + 1) * tc)
            yeng.tensor_tensor(yt[:, sl], h_t[:, sl], cb2, OP.mult)
        flush_acc(n - LAG)
        flush_carry(n - LAG)
    for m in range(NST - LAG, NST):
        flush_acc(m)
        flush_carry(m)
    if not USE_ACCUM_DMA:
        lv = [yts[n] for n in range(NST)]
        rot = 0
        while len(lv) > 1:
            nx = []
            for i in range(0, len(lv) - 1, 2):
                o = c.ytp.tile([128, t2], BF16, tag="yt", name="yt")
                eng = nc.gpsimd if rot % 3 == 2 else nc.vector
                eng.tensor_tensor(o, lv[i], lv[i + 1], OP.add)
                nx.append(o)
                rot += 1
            if len(lv) % 2:
                nx.append(lv[-1])
            lv = nx
        c.y_sum = lv[0]


def _outproj(c, nc, l, xout, t0, yg, blk):
    bw = c.bw
    ps = c.pp.tile([128, bw], F32, tag="mm", name="mm")
    for h in range(bw // 512):
        col = blk * bw + h * 512
        for ct in range(2):
            nc.tensor.matmul(
                ps[:, h * 512:(h + 1) * 512],
                lhsT=c.w_ow[l][:, ct * 128:ct * 128 + 128],
                rhs=yg[:, ct * c.tc_len + col:ct * c.tc_len + col + 512],
                start=(ct == 0), stop=(ct == 1))
    nc.scalar.activation(
        xout[:, 3 + t0 + blk * bw:3 + t0 + (blk + 1) * bw], ps, AF.Copy)


def _layer(c, nc, l, bcd):
    xin = c.xa if l % 2 == 0 else c.xb
    xout = c.xb if l % 2 == 0 else c.xa
    t2 = 2 * c.tc_len
    for ci in range(c.nch):
        t0 = ci * c.tc_len
        c.u_sb = c.ubufp.tile([128, t2], BF16, tag="u", name="u")
        c.zs_sb = c.zbufp.tile([128, t2], BF16, tag="z", name="z")
        c.de_sb = c.dbufp.tile([128, t2], BF16, tag="de", name="de")
        c.du_sb = c.dubufp.tile([128, t2], BF16, tag="du", name="du")
        c.y_sb = c.ybufp.tile([128, t2], BF16, tag="y", name="y")
        c.xd_sb = c.xdblp.tile([40, c.tc_len], BF16, tag="xd", name="xd")

        for db in range(2):
            for blk in range(c.nblk):
                _uz(c, nc, l, xin, t0, db, blk)
        for blk in range(c.nblk):
            _xdbl(c, nc, l, blk)
        for db in range(2):
            for blk in range(c.nblk):
                _delta(c, nc, l, db, blk)
        nc.gpsimd.tensor_tensor(c.du_sb, c.de_sb, c.u_sb, OP.mult)

        _ssm(c, nc, l, ci, bcd[(l * c.nch + ci) % 4])

        ysum = c.y_sb if USE_ACCUM_DMA else c.y_sum
        yg = c.ygatep.tile([128, t2], BF16, tag="yg", name="yg")
        yf = c.ygatep.tile([128, t2], BF16, tag="yf", name="yf")
        for db in range(2):
            sl = slice(db * c.tc_len, (db + 1) * c.tc_len)
            # yf = (u * D) - S'  (S' = -sum_n C h)
            nc.vector.scalar_tensor_tensor(
                yf[:, sl], c.u_sb[:, sl], c.w_d[l][:, db:db + 1],
                ysum[:, sl], OP.mult, OP.subtract)
        nc.vector.tensor_tensor(yg, yf, c.zs_sb, OP.mult)
        for blk in range(c.nblk):
            _outproj(c, nc, l, xout, t0, yg, blk)


def _head(c, nc, out):
    xfin = c.xa if NL % 2 == 0 else c.xb
    for blk in range(c.s // c.bw):
        ps = c.pp.tile([128, c.bw], F32, tag="mm", name="mm")
        for h in range(c.bw // 512):
            col = blk * c.bw + h * 512
            nc.tensor.matmul(
                ps[0:1, h * 512:(h + 1) * 512],
                lhsT=c.w_headw, rhs=xfin[:, 3 + col:3 + col + 512],
                start=True, stop=True)
        ot = c.tmpp.tile([128, c.bw], F32, tag="tm", name="ot")
        nc.scalar.activation(ot[0:1, :], ps[0:1, :], AF.Sigmoid,
                             bias=c.w_headb[0:1, 0:1])
        nc.sync.dma_start(out[0:1, blk * c.bw:(blk + 1) * c.bw], ot[0:1, :])


def build(s=S, tc_len=1024, nloops=1):
    nc = bacc.Bacc("TRN2", target_bir_lowering=False, debug=False,
                   num_devices=NCORES)
    drams = (
        nc.declare_dram_parameter("kuc", [NL, 128, DC * DI], BF16, False),
        nc.declare_dram_parameter("wz", [NL, 128, DI], BF16, False),
        nc.declare_dram_parameter("xpw", [NL, 128, 80], BF16, False),
        nc.declare_dram_parameter("dtw", [NL, RK, DI], BF16, False),
        nc.declare_dram_parameter("ow", [NL, 128, 256], BF16, False),
        nc.declare_dram_parameter("emb", [11, DM], BF16, False),
        nc.declare_dram_parameter("headw", [DM, 1], BF16, False),
        nc.declare_dram_parameter("dtbn", [NL, 128, 2], F32, False),
        nc.declare_dram_parameter("cb", [NL, 128, 2], F32, False),
        nc.declare_dram_parameter("apos", [NL, 128, 2 * NST], F32, False),
        nc.declare_dram_parameter("dpar", [NL, 128, 2], F32, False),
        nc.declare_dram_parameter("embb", [128, 1], F32, False),
        nc.declare_dram_parameter("headb", [1, 1], F32, False),
    )
    featT = nc.declare_dram_parameter("featT", [11, s], BF16, False)
    out = nc.declare_dram_parameter("out", [1, s], F32, True)
    bcd = nc.dram_tensor("bcd", [4, 2 * NST, tc_len], BF16)
    drams = tuple(list(drams) + [featT])

    c = Ctx()
    c.s = s
    c.tc_len = tc_len
    c.nch = s // tc_len
    c.bw = min(tc_len, 1024)
    c.nblk = tc_len // c.bw
    c.da_bufs = 2
    c.da_seen = 0

    with tile.TileContext(nc) as tcx:
        with (
            tcx.tile_pool(name="w", bufs=1) as wp,
            tcx.tile_pool(name="psP", bufs=4, space="PSUM") as pp,
            tcx.tile_pool(name="bcast", bufs=6) as bcp,
        ):
            c.wp, c.pp, c.bcp = wp, pp, bcp
            _load_weights(c, nc, drams)
            c.xa = wp.tile([128, 3 + s], BF16, tag="xa", name="xa")
            c.xb = wp.tile([128, 3 + s], BF16, tag="xb", name="xb")
            nc.vector.memset(c.xa[:, 0:3], 0.0)
            nc.vector.memset(c.xb[:, 0:3], 0.0)
            # per-n carry tiles, double-banked across layers (l % 2)
            c.hc = [[wp.tile([128, 2], F32, tag=f"hc{b}_{n}",
                             name=f"hc{b}_{n}") for n in range(NST)]
                    for b in range(2)]

            with tcx.tile_pool(name="feat", bufs=1) as fp:
                c.w_feat = fp.tile([11, s], BF16, tag="featT", name="featT")
                nc.sync.dma_start(c.w_feat, drams[13][:])
                _embed(c, nc)

            with (
                tcx.tile_pool(name="ubuf", bufs=2) as ubufp,
                tcx.tile_pool(name="zbuf", bufs=2) as zbufp,
                tcx.tile_pool(name="dbuf", bufs=2) as dbufp,
                tcx.tile_pool(name="dubuf", bufs=2) as dubufp,
                tcx.tile_pool(name="xdbl", bufs=2) as xdblp,
                tcx.tile_pool(name="ybuf", bufs=2) as ybufp,
                tcx.tile_pool(name="ygate", bufs=1) as ygatep,
                tcx.tile_pool(name="da", bufs=2) as dap,
                tcx.tile_pool(name="dbu", bufs=2) as dbup,
                tcx.tile_pool(name="hb", bufs=5) as hp,
                tcx.tile_pool(name="yt", bufs=6) as ytp,
                tcx.tile_pool(name="tmp", bufs=2) as tmpp,
            ):
                c.ubufp, c.zbufp, c.dbufp, c.dubufp = ubufp, zbufp, dbufp, dubufp
                c.xdblp, c.ybufp, c.ygatep = xdblp, ybufp, ygatep
                c.dap, c.dbup, c.hp, c.ytp, c.tmpp = dap, dbup, hp, ytp, tmpp

                for rep in range(nloops):
                    for l in range(NL):
                        _layer(c, nc, l, bcd)
                _head(c, nc, out)

    nc.compile()
    return nc


_CACHE = {}


def _get_nc(s, tc_len, nloops=1):
    key = (s, tc_len, nloops)
    if key not in _CACHE:
        _CACHE[key] = build(s, tc_len, nloops)
    return _CACHE[key]


def prep_inputs(features, emb_w, emb_b, in_proj_w, conv_w, conv_b, x_proj_w,
                dt_w, dt_b, A_log, D, out_proj_w, head_w, head_b):
    """Host-side weight preprocessing shared by all cores."""
    import ml_dtypes
    f32 = np.float32
    bf16 = ml_dtypes.bfloat16

    nl = in_proj_w.shape[0]
    kuc = np.zeros((nl, 128, DC * DI), dtype=f32)
    for l in range(nl):
        wu = in_proj_w[l][:, :DI]                      # [128, 256]
        for k in range(DC):
            kuc[l][:, k * DI:(k + 1) * DI] = wu * conv_w[l][:, k][None, :]
    wz = in_proj_w[:, :, DI:]                          # [NL, 128, 256]
    xpw = np.zeros((nl, 128, 80), dtype=f32)
    ow = np.zeros((nl, 128, 256), dtype=f32)
    apos = np.zeros((nl, 128, 2 * NST), dtype=f32)
    dtbn = np.zeros((nl, 128, 2), dtype=f32)
    cb2 = np.zeros((nl, 128, 2), dtype=f32)
    dp2 = np.zeros((nl, 128, 2), dtype=f32)
    for l in range(nl):
        for ct in range(2):
            xpw[l][:, ct * 40:(ct + 1) * 40] = \
                x_proj_w[l][ct * 128:(ct + 1) * 128, :]
            ow[l][:, ct * 128:(ct + 1) * 128] = \
                out_proj_w[l][ct * 128:(ct + 1) * 128, :]
            apos[l][:, ct * NST:(ct + 1) * NST] = \
                np.exp(A_log[l][ct * 128:(ct + 1) * 128, :])
            dtbn[l][:, ct] = -dt_b[l][ct * 128:(ct + 1) * 128]
            cb2[l][:, ct] = conv_b[l][ct * 128:(ct + 1) * 128]
            dp2[l][:, ct] = D[l][ct * 128:(ct + 1) * 128]

    return {
        "kuc": kuc.astype(bf16),
        "wz": np.ascontiguousarray(wz).astype(bf16),
        "xpw": xpw.astype(bf16),
        "dtw": np.ascontiguousarray(dt_w).astype(bf16),
        "ow": ow.astype(bf16),
        "emb": np.ascontiguousarray(emb_w).astype(bf16),
        "headw": np.ascontiguousarray(head_w).astype(bf16),
        "dtbn": dtbn,
        "cb": cb2,
        "apos": apos,
        "dpar": dp2,
        "embb": np.asarray(emb_b).reshape(128, 1).astype(f32),
        "headb": np.asarray(head_b).reshape(1, 1).astype(f32),
    }


def kernel(features, emb_w, emb_b, in_proj_w, conv_w, conv_b, x_proj_w,
           dt_w, dt_b, A_log, D, out_proj_w, head_w, head_b,
           _tc_len=1024, _trace=False):
    from concourse.bass_utils import run_bass_kernel_spmd
    import ml_dtypes

    args = [np.asarray(a) for a in (
        features, emb_w, emb_b, in_proj_w, conv_w, conv_b, x_proj_w,
        dt_w, dt_b, A_log, D, out_proj_w, head_w, head_b)]
    features = args[0]
    b, s, _ = features.shape
    assert b == NCORES
    nc = _get_nc(s, _tc_len)
    common = prep_inputs(*args)
    in_maps = []
    for i in range(NCORES):
        m = dict(common)
        m["featT"] = np.ascontiguousarray(
            features[i].T).astype(ml_dtypes.bfloat16)
        in_maps.append(m)
    res = run_bass_kernel_spmd(nc, in_maps, core_ids=list(range(NCORES)),
                               trace=_trace)
    out = np.stack([r["out"].reshape(s, 1) for r in res.results])
    kernel.last_result = res
    return out.astype(np.float32)
